# revision 3
# baseline (speedup 1.0000x reference)
"""Trainium2 Bass kernel for nn_NeuralCA_19696720019747.

Neural CA: 40 steps of
    p  = conv3x3(x, w_percep)            # 16 -> 48, SAME pad
    h1 = relu(w1 @ p + b1)               # 48 -> 128 (1x1)
    h2 = relu(w2 @ h1 + b2)              # 128 -> 128
    dx = w3 @ h2                         # 128 -> 16
    x  = clip(x + dx, 0, 1)
on x [4, 16, 224, 224] fp32.

Strategy (8 NeuronCores, pure data parallel, zero inter-core comm):
- Shard (batch x image-half): core m owns image m//2, rows [0..111] or
  [112..223]. Bottom halves are vertically flipped on the host so every
  core runs the identical program with its cut edge "below"; the conv dy
  taps are flipped in the weights for those cores.
- Shrinking-halo: each core holds its 112 rows + 40 halo rows. Step s
  computes rows 0..150-s; garbage creeps up 1 row/step from the bottom
  and never reaches the owned rows. No collectives.
- w1 is folded into the conv on the host (conv and 1x1 compose), so the
  perception conv directly produces the 128-wide hidden pre-activation:
  3 matmuls (one per dy) of contraction 48 = (16 ch x 3 dx-shifted
  copies) over a dx-packed staging buffer, accumulating in PSUM.
- Pixels are matmul moving columns (fp32r = full-rate fp32 matmul mode);
  state x stays in SBUF in full fp32 for all 40 steps; the dx-packed
  conv input is staged in chunks via SBUF->SBUF DMA (fp32r-rounded).
- Engines: TensorE 5 matmuls/tile, ACT both relus (per-partition bias
  fused), DVE the +dx accumulate (PSUM read), GPSIMD the clip.
"""

import math

import numpy as np

CH = 16
HID = 128
B, H, W = 4, 224, 224
WP = W + 2              # 226: one zero pad col each side
OWN = 112               # rows owned per core
HALO = 40
DATA_ROWS = OWN + HALO  # 152 local rows
BUF_ROWS = DATA_ROWS + 2  # 154: zero row above + spare below
FLAT = BUF_ROWS * WP    # 34804
CHUNK = 16              # output rows staged per y48 chunk
N_CORES = 8

_cache = {}


def _build(steps: int):
    import concourse.bass as bass  # noqa: F401
    import concourse.tile as tile
    import concourse.mybir as mybir
    from concourse import bacc

    f32 = mybir.dt.float32
    f32r = mybir.dt.float32r
    Relu = mybir.ActivationFunctionType.Relu

    nc = bacc.Bacc("TRN2", target_bir_lowering=False, debug=False,
                   enable_asserts=False)

    x_d = nc.dram_tensor("x_pad", [CH, FLAT], f32, kind="ExternalInput").ap()
    wa_d = nc.dram_tensor("wa", [48, 3 * HID], f32, kind="ExternalInput").ap()
    w2_d = nc.dram_tensor("w2t", [HID, HID], f32, kind="ExternalInput").ap()
    w3_d = nc.dram_tensor("w3t", [HID, CH], f32, kind="ExternalInput").ap()
    b1_d = nc.dram_tensor("b1c", [HID, 1], f32, kind="ExternalInput").ap()
    b2_d = nc.dram_tensor("b2c", [HID, 1], f32, kind="ExternalInput").ap()
    out_d = nc.dram_tensor("out", [CH, OWN * W], f32, kind="ExternalOutput").ap()

    with tile.TileContext(nc) as tc:
        xs_t = nc.alloc_sbuf_tensor("xs", [CH, FLAT], f32)
        wa_t = nc.alloc_sbuf_tensor("wa_s", [48, 3 * HID], f32r)
        w2_t = nc.alloc_sbuf_tensor("w2_s", [HID, HID], f32r)
        w3_t = nc.alloc_sbuf_tensor("w3_s", [HID, CH], f32r)
        b1_t = nc.alloc_sbuf_tensor("b1_s", [HID, 1], f32)
        b2_t = nc.alloc_sbuf_tensor("b2_s", [HID, 1], f32)

        xs = xs_t.ap()
        xsr = xs.rearrange("p (r w) -> p r w", w=WP)  # [16, 154, 226]
        wa = wa_t.ap()
        w2 = w2_t.ap()
        w3 = w3_t.ap()
        b1 = b1_t.ap()[:, 0:1]
        b2 = b2_t.ap()[:, 0:1]

        nc.sync.dma_start(out=xs, in_=x_d)
        nc.sync.dma_start(out=wa, in_=wa_d.bitcast(f32r))
        nc.sync.dma_start(out=w2, in_=w2_d.bitcast(f32r))
        nc.sync.dma_start(out=w3, in_=w3_d.bitcast(f32r))
        nc.sync.dma_start(out=b1_t.ap(), in_=b1_d)
        nc.sync.dma_start(out=b2_t.ap(), in_=b2_d)

        YCOLS = (CHUNK + 2) * WP  # staging tile cols

        with tc.tile_pool(name="ypool", bufs=3) as ypool, \
             tc.tile_pool(name="h1pool", bufs=3) as h1pool, \
             tc.tile_pool(name="h2pool", bufs=3) as h2pool, \
             tc.tile_pool(name="pc", bufs=2, space="PSUM") as pcpool, \
             tc.tile_pool(name="p2", bufs=2, space="PSUM") as p2pool, \
             tc.tile_pool(name="p3", bufs=2, space="PSUM") as p3pool:

            xr = xs.bitcast(f32r)

            def build_chunk(r0, rows_out):
                """Stage dx-packed y48 rows r0-1..r0+rows_out into a pool tile.

                Must be emitted BEFORE the updates that overwrite the overlap
                rows (r0-1 and r0+rows_out), i.e. prefetched one chunk ahead.
                """
                nstage = rows_out + 2
                flat0 = r0 * WP            # buffer row r0 == local row r0-1
                L = nstage * WP
                yt = ypool.tile([48, YCOLS], f32r, tag="y")
                # y[16*dxi + ch, j] = x[ch, flat0 + j + (dxi-1)]
                nc.sync.dma_start(out=yt[0:16, 1:L], in_=xr[:, flat0:flat0 + L - 1])
                nc.sync.dma_start(out=yt[16:32, 0:L], in_=xr[:, flat0:flat0 + L])
                nc.sync.dma_start(out=yt[32:48, 0:L - 1], in_=xr[:, flat0 + 1:flat0 + L])
                return yt

            for s in range(steps):
                R = (DATA_ROWS - 1) - s  # output rows this step
                nchunks = math.ceil(R / CHUNK)
                yt = build_chunk(0, min(CHUNK, R))
                for c in range(nchunks):
                    r0 = CHUNK * c
                    rows_out = min(CHUNK, R - r0)
                    # prefetch next chunk's staging before this chunk's updates
                    # overwrite the overlap rows
                    if c + 1 < nchunks:
                        yt_next = build_chunk(CHUNK * (c + 1),
                                              min(CHUNK, R - CHUNK * (c + 1)))
                    else:
                        yt_next = None

                    for k4 in range(0, rows_out, 4):
                        sub = []  # (bank, nrows, ncols, out_row_base)
                        for bk in range(2):
                            rr = r0 + k4 + 2 * bk
                            nr = min(2, R - rr)
                            if nr > 0:
                                sub.append((bk, nr, nr * WP, rr))
                        pc = pcpool.tile([HID, 1024], f32, tag="pc")
                        # conv (+ fused w1): 3 dy matmuls accumulate per bank
                        for dyi in range(3):
                            for bk, nr, ncols, rr in sub:
                                ro = k4 + 2 * bk  # row offset inside chunk
                                nc.tensor.matmul(
                                    pc[:, 512 * bk:512 * bk + ncols],
                                    lhsT=wa[:, HID * dyi:HID * (dyi + 1)],
                                    rhs=yt[:, (ro + dyi) * WP:(ro + dyi) * WP + ncols],
                                    start=(dyi == 0), stop=(dyi == 2))
                        h1 = h1pool.tile([HID, 904], f32r, tag="h1")
                        if len(sub) == 2 and sub[0][1] == 2 and sub[1][1] == 2:
                            nc.scalar.activation(
                                h1.rearrange("p (b x) -> p b x", b=2),
                                pc.rearrange("p (b x) -> p b x", b=2)[:, :, 0:452],
                                Relu, bias=b1)
                        else:
                            for bk, nr, ncols, rr in sub:
                                nc.scalar.activation(
                                    h1[:, 452 * bk:452 * bk + ncols],
                                    pc[:, 512 * bk:512 * bk + ncols],
                                    Relu, bias=b1)
                        for bk, nr, ncols, rr in sub:
                            p2 = p2pool.tile([HID, 512], f32, tag="p2")
                            nc.tensor.matmul(p2[:, 0:ncols], lhsT=w2,
                                             rhs=h1[:, 452 * bk:452 * bk + ncols],
                                             start=True, stop=True)
                            h2 = h2pool.tile([HID, 452], f32r, tag="h2")
                            nc.scalar.activation(h2[:, 0:ncols], p2[:, 0:ncols],
                                                 Relu, bias=b2)
                            p3 = p3pool.tile([CH, 512], f32, tag="p3")
                            nc.tensor.matmul(p3[:, 0:ncols], lhsT=w3,
                                             rhs=h2[:, 0:ncols],
                                             start=True, stop=True)
                            # x = clip(x + dx) on rows rr..rr+nr-1, pads skipped
                            xv = xsr[:, rr + 1:rr + 1 + nr, 1:1 + W]
                            dxv = p3[:, 0:452].rearrange(
                                "p (r w) -> p r w", w=WP)[:, 0:nr, 1:1 + W]
                            nc.vector.tensor_tensor(
                                out=xv, in0=xv, in1=dxv,
                                op=mybir.AluOpType.add)
                            nc.gpsimd.tensor_scalar(
                                out=xv, in0=xv, scalar1=1.0, scalar2=0.0,
                                op0=mybir.AluOpType.min,
                                op1=mybir.AluOpType.max)
                    yt = yt_next

            nc.sync.dma_start(
                out=out_d.rearrange("p (r w) -> p r w", w=W),
                in_=xsr[:, 1:1 + OWN, 1:1 + W])

    nc.compile()
    return nc


def _prep_inputs(x, w_percep, w1, b1, w2, b2, w3):
    """Host-side: shard, flip bottom halves, fuse w1 into the conv."""
    x = np.asarray(x, np.float32)
    wp = np.asarray(w_percep, np.float32)   # [48, 16, 3, 3] OIHW
    w1 = np.asarray(w1, np.float32)         # [128, 48]
    w2t = np.ascontiguousarray(np.asarray(w2, np.float32).T)  # [128,128]
    w3t = np.ascontiguousarray(np.asarray(w3, np.float32).T)  # [128,16]
    b1c = np.ascontiguousarray(np.asarray(b1, np.float32).reshape(HID, 1))
    b2c = np.ascontiguousarray(np.asarray(b2, np.float32).reshape(HID, 1))

    def fuse(wp_loc):
        blocks = []
        for dyi in range(3):
            m = wp_loc[:, :, dyi, :]                 # [48m, 16c, 3dxi]
            f = np.einsum("om,mcd->dco", w1, m)      # [3, 16, 128]
            blocks.append(f.reshape(48, HID))
        return np.ascontiguousarray(np.concatenate(blocks, axis=1))  # [48, 384]

    wa_top = fuse(wp)
    wa_bot = fuse(wp[:, :, ::-1, :])

    in_maps = []
    for core in range(N_CORES):
        b_, half = core // 2, core % 2
        img = x[b_]
        local = img[:, 0:DATA_ROWS, :] if half == 0 \
            else img[:, ::-1, :][:, 0:DATA_ROWS, :]
        buf = np.zeros((CH, BUF_ROWS, WP), np.float32)
        buf[:, 1:1 + DATA_ROWS, 1:1 + W] = local
        in_maps.append({
            "x_pad": np.ascontiguousarray(buf.reshape(CH, FLAT)),
            "wa": wa_top if half == 0 else wa_bot,
            "w2t": w2t, "w3t": w3t, "b1c": b1c, "b2c": b2c,
        })
    return in_maps


def _gather(results):
    out = np.zeros((B, CH, H, W), np.float32)
    for core in range(N_CORES):
        b_, half = core // 2, core % 2
        o = results[core]["out"].reshape(CH, OWN, W)
        if half == 0:
            out[b_, :, 0:OWN, :] = o
        else:
            out[b_, :, OWN:2 * OWN, :] = o[:, ::-1, :]
    return out


def kernel(x, w_percep, w1, b1, w2, b2, w3, steps, _trace=False):
    import concourse.bass_utils as bass_utils

    steps = int(steps)
    if steps not in _cache:
        _cache[steps] = _build(steps)
    nc = _cache[steps]

    in_maps = _prep_inputs(x, w_percep, w1, b1, w2, b2, w3)
    res = bass_utils.run_bass_kernel_spmd(
        nc, in_maps, core_ids=list(range(N_CORES)), trace=_trace)
    out = _gather(res.results)
    if _trace:
        kernel.last_result = res
    return out
